# revision 3
# baseline (speedup 1.0000x reference)
"""Trainium2 Bass kernel for nn_Coupling: out[e, s*J+j] = sum_a feat[e, a*S+s] * P[a, j].

Sharding: env axis data-parallel across 8 cores (3750 envs/core); P is tiny and
built host-side, replicated to every core as a [108, 30] block-diagonal input.

Per-core device kernel:
  - K-packs 3 envs into one matmul contraction (K = 3*36 = 108 partitions),
    with features as the stationary operand and the 3-env block-diagonal P as
    the 30-column moving operand.
  - For each env-triple, G matmuls: matmul r uses feature columns s = r (mod G),
    so output partition m holds s = G*m + r.  Per partition the (s_lo, j) block
    is G*10 floats contiguous and 64B-aligned in DRAM, keeping the output DMA
    efficient despite the s-major/j-minor interleaved output layout.
  - 128/(S/G) triples share one PSUM batch (col-tiling); one DVE copy per batch
    permutes (r,t,j) -> (t,r,j) into an SBUF staging buffer; one large DMA per
    partition-quarter flushes each stage.
"""

import math
import time

import numpy as np

import concourse.mybir as mybir
from concourse import bacc, tile
from concourse.bass_utils import run_bass_kernel_spmd

ENV = 30000
A = 36          # n_alpha
S = 256         # soap
J = 10          # n_j
N_CORES = 8
E_CORE = ENV // N_CORES  # 3750

T = 3           # envs packed into one matmul contraction (K = T*A = 108)
G = 8           # s values interleaved per output partition (run = G*J*4 bytes)
NT_LD = 4       # triples per feature-load DMA
NB = 8          # PSUM batches per output stage

F32 = mybir.dt.float32

_NC_CACHE = {}


def build_nc(n_env, g=G, fbufs=12, stbufs=2, psbufs=8, dma_only=False):
    assert n_env % T == 0
    n_tri = n_env // T
    part = S // g           # output partitions per triple
    nt_ps = 128 // part     # triples per PSUM batch
    st_tri = NB * nt_ps     # triples per stage

    nc = bacc.Bacc("TRN2", target_bir_lowering=False, debug=False)

    feat = nc.dram_tensor("features", [n_env, A * S], F32, kind="ExternalInput")
    pblk = nc.dram_tensor("pblk", [T * A, T * J], F32, kind="ExternalInput")
    out = nc.dram_tensor("out", [n_env, S * J], F32, kind="ExternalOutput")

    feat3 = feat.rearrange("e (a s) -> e a s", a=A)
    out3 = out.rearrange("e (sh x) -> e sh x", sh=part)  # x = s_lo*J + j

    with tile.TileContext(nc) as tc:
        with (
            tc.tile_pool(name="const", bufs=1) as cpool,
            tc.tile_pool(name="feat", bufs=fbufs) as fpool,
            tc.tile_pool(name="psum", bufs=psbufs, space="PSUM") as pspool,
            tc.tile_pool(name="stage", bufs=stbufs) as stpool,
        ):
            pb = cpool.tile([T * A, T * J], F32)
            nc.sync.dma_start(pb[:], pblk[:])
            dummy = None
            if dma_only:
                dummy = cpool.tile([128, NB, T, g * J], F32)
                nc.gpsimd.memset(dummy[:], 0.0)

            tri0 = 0
            while tri0 < n_tri:
                n_tri_st = min(st_tri, n_tri - tri0)
                n_grp = math.ceil(n_tri_st / NT_LD)
                e0 = tri0 * T

                # load feature groups (NT_LD consecutive triples each)
                fts = []
                for gi in range(n_grp):
                    nt = min(NT_LD, n_tri_st - gi * NT_LD)
                    eg = e0 + gi * NT_LD * T
                    ft = fpool.tile([T * A, nt, S], F32)
                    nc.sync.dma_start(
                        ft[:],
                        feat3[eg : eg + nt * T].rearrange(
                            "(m t) a s -> t a m s", t=T
                        ),
                    )
                    fts.append(ft.rearrange("p m (sh g) -> p m g sh", g=g))

                if not dma_only:
                    stage = stpool.tile([128, NB, T, g * J], F32)
                    # triple tau -> quarter q = tau//NB, psum batch b = tau%NB
                    for b in range(min(NB, n_tri_st)):
                        nq = sum(1 for q in range(nt_ps) if NB * q + b < n_tri_st)
                        ps = pspool.tile([128, g, T, J], F32)
                        for q in range(nq):
                            tau = NB * q + b
                            gi, mm = divmod(tau, NT_LD)
                            for r in range(g):
                                nc.tensor.matmul(
                                    ps[q * part : (q + 1) * part, r],
                                    fts[gi][:, mm, r],
                                    pb[:],
                                    tile_position=(0, q * part),
                                )
                        nc.vector.tensor_copy(
                            stage[: nq * part, b].rearrange(
                                "p t (r j) -> p t r j", r=g
                            ),
                            ps[: nq * part].rearrange("p r t j -> p t r j"),
                        )
                else:
                    stage = dummy

                # flush stage: quarter q covers triples [NB*q, NB*q+NB)
                for q in range(math.ceil(n_tri_st / NB)):
                    nb_q = min(NB, n_tri_st - NB * q)
                    eq = e0 + NB * q * T
                    nc.scalar.dma_start(
                        out3[eq : eq + nb_q * T].rearrange("e sh x -> sh e x"),
                        stage[q * part : (q + 1) * part, :nb_q],
                    )

                tri0 += n_tri_st

    nc.compile()
    return nc


def _get_nc(n_env, **kw):
    key = (n_env, tuple(sorted(kw.items())))
    if key not in _NC_CACHE:
        _NC_CACHE[key] = build_nc(n_env, **kw)
    return _NC_CACHE[key]


def make_pblk(U, alpha1, alpha2, j1, j2):
    P = (U[alpha1][:, j1] * U[alpha2][:, j2]).astype(np.float32)  # [A, J]
    pblk = np.zeros((T * A, T * J), dtype=np.float32)
    for t in range(T):
        pblk[t * A : (t + 1) * A, t * J : (t + 1) * J] = P
    return pblk


def run_spmd(features, U, alpha1, alpha2, j1, j2, trace=False, **kw):
    features = np.asarray(features, dtype=np.float32)
    pblk = make_pblk(
        np.asarray(U), np.asarray(alpha1), np.asarray(alpha2),
        np.asarray(j1), np.asarray(j2),
    )
    nc = _get_nc(E_CORE, **kw)
    in_maps = [
        {"features": features[c * E_CORE : (c + 1) * E_CORE], "pblk": pblk}
        for c in range(N_CORES)
    ]
    res = run_bass_kernel_spmd(nc, in_maps, list(range(N_CORES)), trace=trace)
    out = np.concatenate([res.results[c]["out"] for c in range(N_CORES)], axis=0)
    return out, res


def kernel(features, U, alpha1, alpha2, j1, j2):
    return run_spmd(features, U, alpha1, alpha2, j1, j2)[0]


def bench(inputs, iters=15, **kw):
    """Time repeated on-device executions of the compiled kernel.

    Returns estimated per-execution wall time in ns (min over iters), with
    inputs resident on device so only dispatch overhead + HW exec is counted.
    """
    import jax
    import numpy as np
    from jax.sharding import Mesh, NamedSharding, PartitionSpec
    from jax.experimental.shard_map import shard_map
    import concourse.mybir as mybir
    from concourse import bass2jax
    from concourse.bass2jax import _bass_exec_p, partition_id_tensor

    features = np.asarray(inputs["features"], dtype=np.float32)
    pblk = make_pblk(
        np.asarray(inputs["U"]), np.asarray(inputs["alpha1"]),
        np.asarray(inputs["alpha2"]), np.asarray(inputs["j1"]),
        np.asarray(inputs["j2"]),
    )
    nc = _get_nc(E_CORE, **kw)
    bass2jax.install_neuronx_cc_hook()

    partition_name = (
        nc.partition_id_tensor.name if nc.partition_id_tensor else None
    )
    in_names, out_names, out_avals, zero_outs = [], [], [], []
    for alloc in nc.m.functions[0].allocations:
        if not isinstance(alloc, mybir.MemoryLocationSet):
            continue
        name = alloc.memorylocations[0].name
        if alloc.kind == "ExternalInput":
            if name != partition_name:
                in_names.append(name)
        elif alloc.kind == "ExternalOutput":
            out_names.append(name)
            shape = tuple(alloc.tensor_shape)
            dtype = mybir.dt.np(alloc.dtype)
            out_avals.append(jax.core.ShapedArray(shape, dtype))
            zero_outs.append(np.zeros(shape, dtype))
    n_params = len(in_names)
    all_in_names = list(in_names) + out_names
    if partition_name is not None:
        all_in_names.append(partition_name)
    donate = tuple(range(n_params, n_params + len(out_avals)))

    def _body(*args):
        operands = list(args)
        if partition_name is not None:
            operands.append(partition_id_tensor())
        outs = _bass_exec_p.bind(
            *operands,
            out_avals=tuple(out_avals),
            in_names=tuple(all_in_names),
            out_names=tuple(out_names),
            lowering_input_output_aliases=(),
            sim_require_finite=True,
            sim_require_nnan=True,
            nc=nc,
        )
        return tuple(outs)

    devices = jax.devices()[:N_CORES]
    mesh = Mesh(np.asarray(devices), ("core",))
    spec = NamedSharding(mesh, PartitionSpec("core"))
    fn = jax.jit(
        shard_map(
            _body, mesh=mesh,
            in_specs=(PartitionSpec("core"),) * (n_params + len(out_avals)),
            out_specs=(PartitionSpec("core"),) * len(out_names),
            check_rep=False,
        ),
        donate_argnums=donate, keep_unused=True,
    )

    # device-resident concatenated inputs (features per-core slices + pblk
    # replicated per core)
    per_core = {
        "features": features.reshape(N_CORES, E_CORE, A * S),
        "pblk": np.broadcast_to(pblk, (N_CORES, *pblk.shape)),
    }
    dev_in = [
        jax.device_put(
            per_core[name].reshape(-1, per_core[name].shape[-1]), spec
        )
        for name in in_names
    ]
    bufs = [
        jax.device_put(
            np.zeros((N_CORES * z.shape[0], *z.shape[1:]), z.dtype), spec
        )
        for z in zero_outs
    ]

    times = []
    for i in range(iters):
        t0 = time.perf_counter()
        outs = fn(*dev_in, *bufs)
        jax.block_until_ready(outs)
        times.append(time.perf_counter() - t0)
        bufs = list(outs)  # donate previous outputs back in
    times_ns = sorted(t * 1e9 for t in times[1:])
    print(f"bench times (us): {[f'{t/1e3:.0f}' for t in times_ns]}")
    return times_ns[0]

